# revision 14
# baseline (speedup 1.0000x reference)
"""Trainium2 Bass kernel for nn_MultiHeadODELinear.

Math: out = sum_{k=0..4} (t^k/k!) blockdiag(A_h)^k (x @ W.T + b)
The Taylor loop commutes with the token dimension, so it folds into the
projection:  out = x @ W_eff.T + b_eff  with
  W_eff = E @ W,  b_eff = E @ b,  E = blockdiag(M_h),
  M_h  = sum_{k=0..4} (t^k/k!) A_h^k   (16 heads of 64x64).

Per-core work (data-parallel over batch, 1 batch of [4096, 1024] per core).
x / W / A are cast to fp16 and x is pre-tiled host-side so that each
128-token tile arrives as one contiguous 256KB DMA with the feature dim on
partitions -- no on-chip transposes.  The device output is fp16 (upcast to
f32 on the host); measured end-to-end rel err vs the f32 reference is
~4.4e-4 (gate 2e-2).

Device schedule:
  phase 0: Horner recurrence for N = blockdiag(M_h^T) (32 matmuls, 128-free,
    chunks batched 4-per-PSUM-bank) + WT_eff[d, o] = sum_m W[m, d] N[m, o]
    exploiting that N is chunk-block-diagonal (64 matmuls, 128-free;
    copybacks split DVE/ACT), + b_eff broadcast.  W_eff is built in two
    half-tiles (o 0:512 / 512:1024), lo first, so the first token tile can
    start before the hi half lands.
  phase 1: per 128-token tile: 16 accumulating fp16 matmuls (8 d-chunks x
    2 psum halves, dc-outer so both matmuls of a chunk share one stationary),
    DVE bias-add doubles as PSUM->SBUF copyback, one 256KB output DMA.

DMA rings: SP ring carries t/A/b + even W chunks then even x tiles; ACT
ring carries odd W chunks then odd x tiles (FIFO per ring => W beats x);
gpsimd SWDGE carries the 32 output-tile DMAs.  Total HBM traffic 18MB/core
(x 8MB + W 2MB + out 8MB) ~= 50us -- well under the PE stream, so the
kernel is tensor-engine bound.

HW-measured (slope over full-body NEFF repeats, 8 cores): the PE sustains
~259 ns per (ldweights + 512-free matmul) pair -- ~2.0 GHz effective --
so the 512-matmul main loop floor is ~133 us/pass; probes show the DVE
drains and all three DMA streams add ~0 on top (fully overlapped), and a
constant-stationary probe shows ldweights reloads are also free.
"""

import sys

for _p in ("/opt/trn_rl_repo",):
    if _p not in sys.path:
        sys.path.insert(0, _p)

import numpy as np

import concourse.bass as bass  # noqa: F401
import concourse.tile as tile
from concourse import bacc, mybir
from concourse import bass_utils
from concourse.masks import make_identity

F32 = mybir.dt.float32
F16 = mybir.dt.float16

B, S, D = 8, 4096, 1024
H, HD = 16, 64
ORDERS = 4
P = 128
NCHUNK = D // P          # 8 chunks of 128 along any 1024 dim
TTILES = S // P          # 32 token tiles per core
N_CORES = 8

_NC_CACHE = {}


def _build_nc(repeats=1, variant=()):
    """Build the bass program.

    repeats: how many times the body runs inside one NEFF.  By default each
    repeat is a FULL pass (phase 0 + 32 token tiles) so a repeat-count slope
    measures true single-execution time; with variant "mainrep" phase 0 runs
    once and only the token-tile loop repeats (marginal-throughput probe).
    """
    variant = set(variant)
    full_rep = "mainrep" not in variant and "no_phase0" not in variant

    nc = bacc.Bacc("TRN2", target_bir_lowering=False, debug=False)
    o_dt = F32 if "o_f32" in variant else F16

    # x pre-tiled host-side: x_d[tt, p, c*P+j] = x[tt*P+j, c*P+p]
    x_d = nc.dram_tensor("x", [TTILES, P, D], F16, kind="ExternalInput").ap()
    w_d = nc.dram_tensor("W", [D, D], F16, kind="ExternalInput").ap()
    b_d = nc.dram_tensor("b", [D], F32, kind="ExternalInput").ap()
    a_d = nc.dram_tensor("A", [H, HD, HD], F16, kind="ExternalInput").ap()
    t_d = nc.dram_tensor("t", [1, 1], F32, kind="ExternalInput").ap()
    o_d = nc.dram_tensor("out", [S, D], o_dt, kind="ExternalOutput").ap()

    noxdma = "noxdma" in variant   # probe: single preloaded x tile
    nodve = "nodve" in variant     # probe: skip bias-add drain (and out DMA)
    noodma = "noodma" in variant   # probe: skip out DMA

    with tile.TileContext(nc) as tc:
        with tc.tile_pool(name="const", bufs=1) as const_pool, \
             tc.tile_pool(name="wsb", bufs=1) as w_pool, \
             tc.tile_pool(name="xin", bufs=8) as x_pool, \
             tc.tile_pool(name="osb", bufs=3) as o_pool, \
             tc.tile_pool(name="ps_small", bufs=2, space="PSUM") as ps_small, \
             tc.tile_pool(name="ps_o", bufs=2, space="PSUM") as ps_o:

            # ---- execution-invariant constants (built once per NEFF) ----
            ident = const_pool.tile([P, P], F32)
            make_identity(nc, ident[:])
            ident_rep = const_pool.tile([P, 4, P], F32)
            for q in range(4):
                nc.vector.tensor_copy(ident_rep[:, q, :], ident[:])
            ones_row = const_pool.tile([1, P], F32)
            nc.vector.memset(ones_row[:], 1.0)
            ones_h = const_pool.tile([1, P], F16)
            nc.vector.tensor_copy(ones_h[:], ones_row[:])

            if noxdma:
                xt0 = x_pool.tile([P, NCHUNK, P], F16, tag="xt", name="xt0")
                nc.sync.dma_start(xt0[:], x_d[0])

            def stage_a(it):
                if noxdma:
                    return xt0
                tt = it % TTILES
                xt = x_pool.tile([P, NCHUNK, P], F16, tag="xt", name="xt")
                eng = nc.sync if it % 2 == 0 else nc.scalar
                eng.dma_start(xt[:], x_d[tt])
                return xt

            def phase0():
                """Build W_eff / b_eff.  Returns (wte_lo, wte_hi, b_bcast)."""
                # tiny consts first on the SP ring so they land immediately
                t_sb = const_pool.tile([1, 1], F32, tag="t_sb")
                nc.sync.dma_start(t_sb[:], t_d[:])
                a_blk = const_pool.tile([P, NCHUNK, P], F16, tag="a_blk")
                nc.vector.memset(a_blk[:], 0.0)
                # A as per-chunk block-diagonal pairs: a_blk[:, c, :] holds
                # A[2c] in [0:64, 0:64] and A[2c+1] in [64:128, 64:128].
                a_v = a_d.rearrange("(hp two) i j -> two i hp j", two=2)
                nc.sync.dma_start(a_blk[0:HD, :, 0:HD], a_v[0])
                nc.sync.dma_start(a_blk[HD:P, :, HD:P], a_v[1])
                b_f32 = const_pool.tile([P, NCHUNK], F32, tag="b_f32")
                nc.sync.dma_start(b_f32[:], b_d.rearrange("(c p) -> p c", p=P))

                # W chunks split across both HWDGE rings, ahead of all x
                # tiles, so W gets full DMA bandwidth at t=0.
                w_sb = w_pool.tile([P, NCHUNK, D], F16, tag="w_sb")
                w_view = w_d.rearrange("(c p) d -> p c d", p=P)
                for c in range(NCHUNK):
                    eng = nc.sync if c % 2 == 0 else nc.scalar
                    eng.dma_start(w_sb[:, c, :], w_view[:, c, :])

                # t coefficient vectors c_k = t^k/k! as [128, 1] per-partition
                ps_tv = ps_small.tile([P, 4, P], F32, tag="ps0", name="ps_tv")
                nc.tensor.matmul(ps_tv[:, 0, 0:1], ones_row[:], t_sb[:],
                                 start=True, stop=True)
                c1 = const_pool.tile([P, 1], F32, tag="c1")
                nc.vector.tensor_copy(c1[:], ps_tv[:, 0, 0:1])
                # c_{k} = c_{k-1} * t / k, fused as (in * c1vec) * (1/k)
                c2 = const_pool.tile([P, 1], F32, tag="c2")
                nc.vector.tensor_scalar(c2[:], c1[:], c1[:], 0.5,
                                        mybir.AluOpType.mult,
                                        mybir.AluOpType.mult)
                c3 = const_pool.tile([P, 1], F32, tag="c3")
                nc.vector.tensor_scalar(c3[:], c2[:], c1[:], 1.0 / 3.0,
                                        mybir.AluOpType.mult,
                                        mybir.AluOpType.mult)
                c4 = const_pool.tile([P, 1], F32, tag="c4")
                nc.vector.tensor_scalar(c4[:], c3[:], c1[:], 0.25,
                                        mybir.AluOpType.mult,
                                        mybir.AluOpType.mult)

                # c_k * I replicated 4x (DVE add operands); c4 I in fp16
                # (it is the first Horner matmul rhs)
                c4I_h = const_pool.tile([P, P], F16, tag="c4I")
                nc.vector.tensor_scalar(c4I_h[:], ident[:], c4[:], None,
                                        mybir.AluOpType.mult)
                cI_rep = []
                for i, ck in enumerate((c1, c2, c3)):
                    ckI = const_pool.tile([P, 4, P], F32, tag=f"cIr{i}")
                    nc.vector.tensor_scalar(ckI[:], ident_rep[:], ck[:], None,
                                            mybir.AluOpType.mult)
                    cI_rep.append(ckI)
                c1I_rep, c2I_rep, c3I_rep = cI_rep

                # Horner: S <- A_c^T S + c_k I, starting from rhs = c4*I.
                # After 4 steps S = blockdiag(M_h^T) restricted to chunk c.
                # Chunks batched 4-per-PSUM-bank so each step needs only two
                # DVE adds instead of eight.
                n_sb = const_pool.tile([P, NCHUNK, P], F16, tag="n_sb")
                s_prev = None
                for step in range(ORDERS):
                    tgt = n_sb if step == ORDERS - 1 else \
                        const_pool.tile([P, NCHUNK, P], F16, tag=f"S{step}",
                                        name=f"S{step}")
                    addI = (c3I_rep, c2I_rep, c1I_rep, ident_rep)[step]
                    for g in range(2):
                        ps_s = ps_small.tile([P, 4, P], F32, tag="ps0",
                                             name="ps_s")
                        for q in range(4):
                            c = g * 4 + q
                            rhs = c4I_h[:] if step == 0 else s_prev[:, c, :]
                            nc.tensor.matmul(ps_s[:, q, :], a_blk[:, c, :],
                                             rhs, start=True, stop=True)
                        nc.vector.tensor_tensor(tgt[:, g * 4:(g + 1) * 4, :],
                                                ps_s[:], addI[:],
                                                mybir.AluOpType.add)
                    s_prev = tgt

                # WT_eff[d, o] = sum_m W[m, d] N[m, o].  N is chunk-block-
                # diagonal: only m-chunk == o-chunk contributes, so each
                # (oc, dc) pair is a single 128-free matmul.  Built as two
                # half-tiles (o 0:512 / 512:1024), lo first, so the main
                # loop's first psum group only waits on the lo half.
                # Copybacks alternate DVE / ACT.
                wte_lo = w_pool.tile([P, NCHUNK, 512], F16, tag="wte_lo")
                wte_hi = w_pool.tile([P, NCHUNK, 512], F16, tag="wte_hi")
                for oc in range(NCHUNK):
                    half = wte_lo if oc < 4 else wte_hi
                    och = oc % 4
                    for g in range(2):
                        ps_w = ps_small.tile([P, 4, P], F32, tag="ps0",
                                             name="ps_w")
                        for q in range(4):
                            dc = g * 4 + q
                            nc.tensor.matmul(
                                ps_w[:, q, :],
                                w_sb[:, oc, dc * P:(dc + 1) * P],
                                n_sb[:, oc, :], start=True, stop=True)
                        dst = half[:, g * 4:(g + 1) * 4, och * P:(och + 1) * P]
                        if g == 0:
                            nc.scalar.mul(dst, ps_w[:], 1.0)
                        else:
                            nc.vector.tensor_copy(dst, ps_w[:])

                # b_eff = N^T b, assembled as a [1, 1024] fp16 row then
                # broadcast to 128 partitions via a rank-1 matmul.
                b_h = const_pool.tile([P, NCHUNK], F16, tag="b_h")
                nc.vector.tensor_copy(b_h[:], b_f32[:])
                b_row = const_pool.tile([1, D], F16, tag="b_row")
                for g in range(2):
                    ps_b = ps_small.tile([P, 4, P], F32, tag="ps0",
                                         name="ps_b")
                    for q in range(4):
                        oc = g * 4 + q
                        nc.tensor.matmul(ps_b[0:1, q, :],
                                         b_h[:, oc:oc + 1], n_sb[:, oc, :],
                                         start=True, stop=True)
                    nc.vector.tensor_copy(b_row[:, g * 512:(g + 1) * 512],
                                          ps_b[0:1, :, :])
                b_bcast = const_pool.tile([P, D], F32, tag="b_bcast")
                for hf in range(2):
                    ps_bb = ps_small.tile([P, 4, P], F32, tag="ps0",
                                          name="ps_bb")
                    nc.tensor.matmul(ps_bb[:], ones_h[:],
                                     b_row[:, hf * 512:(hf + 1) * 512],
                                     start=True, stop=True)
                    nc.vector.tensor_copy(
                        b_bcast[:, hf * 512:(hf + 1) * 512], ps_bb[:])
                return wte_lo, wte_hi, b_bcast

            def make_stage_b(wte_half, b_bcast):
                def stage_b(it, xt, first):
                    tt = it % TTILES
                    o_sb = o_pool.tile([P, D], o_dt, name="o_sb")
                    ps = [ps_o.tile([P, 512], F32, tag=f"ps_out{oh}",
                                    name=f"ps_out{oh}") for oh in range(2)]
                    # dc-outer so the two matmuls of each dc share one
                    # stationary (same xt chunk) back to back.  The first
                    # tile of a pass runs oh-outer instead: its first psum
                    # group then only waits on wte_lo, built before wte_hi.
                    if first:
                        for oh in range(2):
                            for dc in range(NCHUNK):
                                nc.tensor.matmul(ps[oh][:], xt[:, dc, :],
                                                 wte_half[oh][:, dc, :],
                                                 start=(dc == 0),
                                                 stop=(dc == NCHUNK - 1))
                    elif "ldprobe" in variant:
                        # timing probe: constant stationary (wrong math)
                        for dc in range(NCHUNK):
                            for oh in range(2):
                                nc.tensor.matmul(ps[oh][:], xt[:, 0, :],
                                                 wte_half[oh][:, dc, :],
                                                 start=(dc == 0),
                                                 stop=(dc == NCHUNK - 1))
                    else:
                        for dc in range(NCHUNK):
                            for oh in range(2):
                                nc.tensor.matmul(ps[oh][:], xt[:, dc, :],
                                                 wte_half[oh][:, dc, :],
                                                 start=(dc == 0),
                                                 stop=(dc == NCHUNK - 1))
                    if nodve:
                        return
                    for oh in range(2):
                        nc.vector.tensor_tensor(
                            o_sb[:, oh * 512:(oh + 1) * 512], ps[oh][:],
                            b_bcast[:, oh * 512:(oh + 1) * 512],
                            mybir.AluOpType.add)
                    if not noodma:
                        nc.gpsimd.dma_start(o_d[tt * P:(tt + 1) * P, :],
                                            o_sb[:])
                return stage_b

            LA = 6  # x-tile DMA lookahead depth (x_pool bufs=8)
            from collections import deque

            def main_loop(n_iters, wte_half, b_bcast, prefetched):
                stage_b = make_stage_b(wte_half, b_bcast)
                q = prefetched
                for i in range(len(q), min(LA, n_iters)):
                    q.append(stage_a(i))
                for it in range(n_iters):
                    if it + LA < n_iters:
                        q.append(stage_a(it + LA))
                    stage_b(it, q.popleft(), first=(it == 0))

            if "no_phase0" in variant:
                wte_lo = w_pool.tile([P, NCHUNK, 512], F16, tag="wte_lo")
                nc.vector.memset(wte_lo[:], 0.0)
                wte_hi = w_pool.tile([P, NCHUNK, 512], F16, tag="wte_hi")
                nc.vector.memset(wte_hi[:], 0.0)
                b_bcast = const_pool.tile([P, D], F32, tag="b_bcast")
                nc.vector.memset(b_bcast[:], 0.0)
                main_loop(TTILES * repeats, (wte_lo, wte_hi), b_bcast,
                          deque())
            elif full_rep:
                for rep in range(repeats):
                    # issue this pass's first x DMAs before the W_eff build
                    # so the ACT-ring copybacks don't delay the x stream
                    prefetched = deque(
                        stage_a(i) for i in range(min(LA, TTILES)))
                    wte_lo, wte_hi, b_bcast = phase0()
                    main_loop(TTILES, (wte_lo, wte_hi), b_bcast, prefetched)
            else:
                prefetched = deque(
                    stage_a(i) for i in range(min(LA, TTILES * repeats)))
                wte_lo, wte_hi, b_bcast = phase0()
                main_loop(TTILES * repeats, (wte_lo, wte_hi), b_bcast,
                          prefetched)

    nc.compile()
    return nc


def get_nc(repeats=1, variant=()):
    key = (repeats, tuple(variant))
    if key not in _NC_CACHE:
        _NC_CACHE[key] = _build_nc(repeats, variant)
    return _NC_CACHE[key]


def make_in_maps(x, t_scalar, W, b, A):
    f16 = np.float16
    x = np.asarray(x, dtype=np.float32)
    # [b, s, d] -> per core [tt, p, c, j] with s = tt*P+j, d = c*P+p
    xt = np.ascontiguousarray(
        x.reshape(B, TTILES, P, NCHUNK, P).transpose(0, 1, 4, 3, 2)
    ).astype(f16).reshape(B, TTILES, P, D)
    t = np.asarray(t_scalar, dtype=np.float32).reshape(1, 1)
    Wb = np.ascontiguousarray(np.asarray(W, dtype=np.float32)).astype(f16)
    b = np.ascontiguousarray(np.asarray(b, dtype=np.float32))
    Ab = np.ascontiguousarray(np.asarray(A, dtype=np.float32)).astype(f16)
    return [{"x": xt[i], "W": Wb, "b": b, "A": Ab, "t": t}
            for i in range(N_CORES)]


def kernel(x, t_scalar, W, b, A):
    nc = get_nc()
    in_maps = make_in_maps(x, t_scalar, W, b, A)
    res = bass_utils.run_bass_kernel_spmd(nc, in_maps,
                                          core_ids=list(range(N_CORES)))
    out = np.stack([res.results[i]["out"] for i in range(N_CORES)], axis=0)
    return np.ascontiguousarray(out.astype(np.float32))


if __name__ == "__main__":
    rng = np.random.default_rng(0)
    x = rng.standard_normal((B, S, D), dtype=np.float32)
    W = rng.standard_normal((D, D), dtype=np.float32) / 32.0
    b = rng.standard_normal((D,), dtype=np.float32) * 0.01
    A = rng.standard_normal((H, HD, HD), dtype=np.float32) * 0.02
    t = np.float32(0.6)
    out = kernel(x, t, W, b, A)
    print("out", out.shape, out.dtype)


# revision 17
# speedup vs baseline: 1.1997x; 1.1997x over previous
"""Trainium2 Bass kernel for nn_MultiHeadODELinear.

Math: out = sum_{k=0..4} (t^k/k!) blockdiag(A_h)^k (x @ W.T + b)
The Taylor loop commutes with the token dimension, so it folds into the
projection:  out = x @ W_eff.T + b_eff  with
  W_eff = E @ W,  b_eff = E @ b,  E = blockdiag(M_h),
  M_h  = sum_{k=0..4} (t^k/k!) A_h^k   (16 heads of 64x64).

Per-core work (data-parallel over batch, 1 batch of [4096, 1024] per core).
x / W / A are cast to fp16 and x is pre-tiled host-side so that each
128-token tile arrives as one contiguous 256KB DMA with the feature dim on
partitions -- no on-chip transposes.  The device output is fp16 (upcast to
f32 on the host); measured end-to-end rel err vs the f32 reference is
~4.4e-4 (gate 2e-2).

Device schedule:
  phase 0: Horner recurrence for N = blockdiag(M_h^T) (32 matmuls, 128-free,
    chunks batched 4-per-PSUM-bank) + WT_eff[d, o] = sum_m W[m, d] N[m, o]
    exploiting that N is chunk-block-diagonal (64 matmuls, 128-free;
    copybacks split DVE/ACT), + b_eff broadcast.  W_eff is built in two
    half-tiles (o 0:512 / 512:1024), lo first, so the first token tile can
    start before the hi half lands.
  phase 1: per 128-token tile: 16 accumulating fp16 matmuls (8 d-chunks x
    2 psum halves, dc-outer so both matmuls of a chunk share one stationary),
    DVE bias-add doubles as PSUM->SBUF copyback, one 256KB output DMA.

DMA rings: SP ring carries t/A/b + even W chunks then even x tiles; ACT
ring carries odd W chunks then odd x tiles (FIFO per ring => W beats x);
gpsimd SWDGE carries the 32 output-tile DMAs.  Total HBM traffic 18MB/core
(x 8MB + W 2MB + out 8MB) ~= 50us -- well under the PE stream, so the
kernel is tensor-engine bound.

HW-measured (slope over full-body NEFF repeats, 8 cores): the PE sustains
~259 ns per (ldweights + 512-free matmul) pair -- ~2.0 GHz effective --
so the 512-matmul main loop floor is ~133 us/pass; probes show the DVE
drains and all three DMA streams add ~0 on top (fully overlapped), and a
constant-stationary probe shows ldweights reloads are also free.
"""

import sys

for _p in ("/opt/trn_rl_repo",):
    if _p not in sys.path:
        sys.path.insert(0, _p)

import numpy as np

import concourse.bass as bass  # noqa: F401
import concourse.tile as tile
from concourse import bacc, mybir
from concourse import bass_utils
from concourse.masks import make_identity

F32 = mybir.dt.float32
F16 = mybir.dt.float16

B, S, D = 8, 4096, 1024
H, HD = 16, 64
ORDERS = 4
P = 128
NCHUNK = D // P          # 8 chunks of 128 along any 1024 dim
TTILES = S // P          # 32 token tiles per core
N_CORES = 8

_NC_CACHE = {}


def _build_nc(repeats=1, variant=()):
    """Build the bass program.

    repeats: how many times the body runs inside one NEFF.  By default each
    repeat is a FULL pass (phase 0 + 32 token tiles) so a repeat-count slope
    measures true single-execution time; with variant "mainrep" phase 0 runs
    once and only the token-tile loop repeats (marginal-throughput probe).
    """
    variant = set(variant)
    full_rep = "mainrep" not in variant and "no_phase0" not in variant

    nc = bacc.Bacc("TRN2", target_bir_lowering=False, debug=False)
    o_dt = F32 if "o_f32" in variant else F16

    # x pre-tiled host-side: x_d[tt, p, c*P+j] = x[tt*P+j, c*P+p]
    x_d = nc.dram_tensor("x", [TTILES, P, D], F16, kind="ExternalInput").ap()
    w_d = nc.dram_tensor("W", [D, D], F16, kind="ExternalInput").ap()
    b_d = nc.dram_tensor("b", [D], F32, kind="ExternalInput").ap()
    a_d = nc.dram_tensor("A", [H, HD, HD], F16, kind="ExternalInput").ap()
    t_d = nc.dram_tensor("t", [1, 1], F32, kind="ExternalInput").ap()
    o_d = nc.dram_tensor("out", [S, D], o_dt, kind="ExternalOutput").ap()

    noxdma = "noxdma" in variant   # probe: single preloaded x tile
    nodve = "nodve" in variant     # probe: skip bias-add drain (and out DMA)
    noodma = "noodma" in variant   # probe: skip out DMA

    with tile.TileContext(nc) as tc:
        with tc.tile_pool(name="const", bufs=1) as const_pool, \
             tc.tile_pool(name="wsb", bufs=1) as w_pool, \
             tc.tile_pool(name="xin", bufs=8) as x_pool, \
             tc.tile_pool(name="osb", bufs=3) as o_pool, \
             tc.tile_pool(name="ps_small", bufs=2, space="PSUM") as ps_small, \
             tc.tile_pool(name="ps_o", bufs=2, space="PSUM") as ps_o:

            # ---- execution-invariant constants (built once per NEFF) ----
            ident = const_pool.tile([P, P], F32)
            make_identity(nc, ident[:])
            ident_rep = const_pool.tile([P, 4, P], F32)
            for q in range(4):
                nc.vector.tensor_copy(ident_rep[:, q, :], ident[:])
            ones_row = const_pool.tile([1, P], F32)
            nc.vector.memset(ones_row[:], 1.0)
            ones_h = const_pool.tile([1, P], F16)
            nc.vector.tensor_copy(ones_h[:], ones_row[:])
            # a_blk's zero padding never changes; only the two DMA'd block
            # regions are rewritten each pass, so memset once here.
            a_blk = const_pool.tile([P, NCHUNK, P], F16, tag="a_blk")
            nc.vector.memset(a_blk[:], 0.0)

            if noxdma:
                xt0 = x_pool.tile([P, NCHUNK, P], F16, tag="xt", name="xt0")
                nc.sync.dma_start(xt0[:], x_d[0])

            def stage_a(it):
                if noxdma:
                    return xt0
                tt = it % TTILES
                xt = x_pool.tile([P, NCHUNK, P], F16, tag="xt", name="xt")
                eng = nc.sync if it % 2 == 0 else nc.scalar
                eng.dma_start(xt[:], x_d[tt])
                return xt

            def phase0():
                """Build W_eff / b_eff.  Returns (wte_lo, wte_hi, b_bcast)."""
                # tiny consts first on the SP ring so they land immediately
                t_sb = const_pool.tile([1, 1], F32, tag="t_sb")
                nc.sync.dma_start(t_sb[:], t_d[:])
                # A as per-chunk block-diagonal pairs: a_blk[:, c, :] holds
                # A[2c] in [0:64, 0:64] and A[2c+1] in [64:128, 64:128].
                a_v = a_d.rearrange("(hp two) i j -> two i hp j", two=2)
                nc.sync.dma_start(a_blk[0:HD, :, 0:HD], a_v[0])
                nc.sync.dma_start(a_blk[HD:P, :, HD:P], a_v[1])
                b_f32 = const_pool.tile([P, NCHUNK], F32, tag="b_f32")
                nc.sync.dma_start(b_f32[:], b_d.rearrange("(c p) -> p c", p=P))

                # W chunks split across both HWDGE rings, ahead of all x
                # tiles, so W gets full DMA bandwidth at t=0.  One tile per
                # chunk so each W_eff band build starts as soon as its own
                # chunk lands.
                w_view = w_d.rearrange("(c p) d -> p c d", p=P)
                w_c = []
                for c in range(NCHUNK):
                    wc = w_pool.tile([P, D], F16, tag=f"w_c{c}")
                    eng = nc.sync if c % 2 == 0 else nc.scalar
                    eng.dma_start(wc[:], w_view[:, c, :])
                    w_c.append(wc)

                # t coefficient vectors c_k = t^k/k! as [128, 1] per-partition
                # (c4 first -- it gates the Horner recurrence)
                ps_tv = ps_small.tile([P, 4, P], F32, tag="ps0", name="ps_tv")
                nc.tensor.matmul(ps_tv[:, 0, 0:1], ones_row[:], t_sb[:],
                                 start=True, stop=True)
                c1 = const_pool.tile([P, 1], F32, tag="c1")
                nc.vector.tensor_copy(c1[:], ps_tv[:, 0, 0:1])
                c2 = const_pool.tile([P, 1], F32, tag="c2")
                nc.vector.tensor_scalar(c2[:], c1[:], c1[:], 0.5,
                                        mybir.AluOpType.mult,
                                        mybir.AluOpType.mult)
                c4 = const_pool.tile([P, 1], F32, tag="c4")
                nc.vector.tensor_scalar(c4[:], c2[:], c2[:], 1.0 / 6.0,
                                        mybir.AluOpType.mult,
                                        mybir.AluOpType.mult)
                # c4 I in fp16: the first Horner matmul rhs
                c4I_h = const_pool.tile([P, P], F16, tag="c4I")
                nc.vector.tensor_scalar(c4I_h[:], ident[:], c4[:], None,
                                        mybir.AluOpType.mult)
                c3 = const_pool.tile([P, 1], F32, tag="c3")
                nc.vector.tensor_scalar(c3[:], c2[:], c1[:], 1.0 / 3.0,
                                        mybir.AluOpType.mult,
                                        mybir.AluOpType.mult)
                cI_rep = []
                for i, ck in enumerate((c3, c2, c1)):
                    ckI = const_pool.tile([P, 4, P], F32, tag=f"cIr{i}")
                    nc.vector.tensor_scalar(ckI[:], ident_rep[:], ck[:], None,
                                            mybir.AluOpType.mult)
                    cI_rep.append(ckI)
                c3I_rep, c2I_rep, c1I_rep = cI_rep

                # Horner: S <- A_c^T S + c_k I, starting from rhs = c4*I.
                # After 4 steps S = blockdiag(M_h^T) restricted to chunk c.
                # Chunks batched 4-per-PSUM-bank; each step's two chunk
                # groups live in separate half tiles so step k+1 of group g
                # only waits on step k of group g (the groups pipeline).
                n_half = [const_pool.tile([P, 4, P], F16, tag=f"n{g}",
                                          name=f"n{g}")
                          for g in range(2)]
                s_prev = None
                for step in range(ORDERS):
                    tgt = n_half if step == ORDERS - 1 else \
                        [const_pool.tile([P, 4, P], F16, tag=f"S{step}g{g}",
                                         name=f"S{step}g{g}")
                         for g in range(2)]
                    addI = (c3I_rep, c2I_rep, c1I_rep, ident_rep)[step]
                    for g in range(2):
                        ps_s = ps_small.tile([P, 4, P], F32, tag="ps0",
                                             name="ps_s")
                        for q in range(4):
                            rhs = c4I_h[:] if step == 0 \
                                else s_prev[g][:, q, :]
                            nc.tensor.matmul(ps_s[:, q, :],
                                             a_blk[:, g * 4 + q, :],
                                             rhs, start=True, stop=True)
                        nc.vector.tensor_tensor(tgt[g][:], ps_s[:], addI[:],
                                                mybir.AluOpType.add)
                    s_prev = tgt

                # WT_eff[d, o] = sum_m W[m, d] N[m, o].  N is chunk-block-
                # diagonal: only m-chunk == o-chunk contributes, so each
                # (oc, dc) pair is a single 128-free matmul.  Built as two
                # half-tiles (o 0:512 / 512:1024), lo first, so the main
                # loop's first psum group only waits on the lo half.
                # Copybacks alternate ACT / DVE.
                wte_lo = w_pool.tile([P, NCHUNK, 512], F16, tag="wte_lo")
                wte_hi = w_pool.tile([P, NCHUNK, 512], F16, tag="wte_hi")
                for oc in range(NCHUNK):
                    half = wte_lo if oc < 4 else wte_hi
                    och = oc % 4
                    n_oc = n_half[oc // 4][:, oc % 4, :]
                    for g in range(2):
                        ps_w = ps_small.tile([P, 4, P], F32, tag="ps0",
                                             name="ps_w")
                        for q in range(4):
                            dc = g * 4 + q
                            nc.tensor.matmul(
                                ps_w[:, q, :],
                                w_c[oc][:, dc * P:(dc + 1) * P],
                                n_oc, start=True, stop=True)
                        dst = half[:, g * 4:(g + 1) * 4, och * P:(och + 1) * P]
                        if g == 0:
                            nc.scalar.mul(dst, ps_w[:], 1.0)
                        else:
                            nc.vector.tensor_copy(dst, ps_w[:])

                # b_eff = N^T b, assembled as a [1, 1024] fp16 row then
                # broadcast to 128 partitions via a rank-1 matmul.
                b_h = const_pool.tile([P, NCHUNK], F16, tag="b_h")
                nc.vector.tensor_copy(b_h[:], b_f32[:])
                b_row = const_pool.tile([1, D], F16, tag="b_row")
                for g in range(2):
                    ps_b = ps_small.tile([P, 4, P], F32, tag="ps0",
                                         name="ps_b")
                    for q in range(4):
                        oc = g * 4 + q
                        nc.tensor.matmul(ps_b[0:1, q, :], b_h[:, oc:oc + 1],
                                         n_half[g][:, q, :],
                                         start=True, stop=True)
                    nc.vector.tensor_copy(b_row[:, g * 512:(g + 1) * 512],
                                          ps_b[0:1, :, :])
                b_bcast = const_pool.tile([P, D], F32, tag="b_bcast")
                for hf in range(2):
                    ps_bb = ps_small.tile([P, 4, P], F32, tag="ps0",
                                          name="ps_bb")
                    nc.tensor.matmul(ps_bb[:], ones_h[:],
                                     b_row[:, hf * 512:(hf + 1) * 512],
                                     start=True, stop=True)
                    nc.vector.tensor_copy(
                        b_bcast[:, hf * 512:(hf + 1) * 512], ps_bb[:])
                return wte_lo, wte_hi, b_bcast

            def make_stage_b(wte_half, b_bcast):
                def stage_b(it, xt, first):
                    tt = it % TTILES
                    o_sb = o_pool.tile([P, D], o_dt, name="o_sb")
                    ps = [ps_o.tile([P, 512], F32, tag=f"ps_out{oh}",
                                    name=f"ps_out{oh}") for oh in range(2)]
                    # dc-outer so the two matmuls of each dc share one
                    # stationary (same xt chunk) back to back.  The first
                    # tile of a pass runs oh-outer instead: its first psum
                    # group then only waits on wte_lo, built before wte_hi.
                    if first:
                        for oh in range(2):
                            for dc in range(NCHUNK):
                                nc.tensor.matmul(ps[oh][:], xt[:, dc, :],
                                                 wte_half[oh][:, dc, :],
                                                 start=(dc == 0),
                                                 stop=(dc == NCHUNK - 1))
                    elif "ldprobe" in variant:
                        # timing probe: constant stationary (wrong math)
                        for dc in range(NCHUNK):
                            for oh in range(2):
                                nc.tensor.matmul(ps[oh][:], xt[:, 0, :],
                                                 wte_half[oh][:, dc, :],
                                                 start=(dc == 0),
                                                 stop=(dc == NCHUNK - 1))
                    else:
                        for dc in range(NCHUNK):
                            for oh in range(2):
                                nc.tensor.matmul(ps[oh][:], xt[:, dc, :],
                                                 wte_half[oh][:, dc, :],
                                                 start=(dc == 0),
                                                 stop=(dc == NCHUNK - 1))
                    if nodve:
                        return
                    for oh in range(2):
                        nc.vector.tensor_tensor(
                            o_sb[:, oh * 512:(oh + 1) * 512], ps[oh][:],
                            b_bcast[:, oh * 512:(oh + 1) * 512],
                            mybir.AluOpType.add)
                    if not noodma:
                        nc.gpsimd.dma_start(o_d[tt * P:(tt + 1) * P, :],
                                            o_sb[:])
                return stage_b

            LA = 6  # x-tile DMA lookahead depth (x_pool bufs=8)
            from collections import deque

            def main_loop(n_iters, wte_half, b_bcast, prefetched):
                stage_b = make_stage_b(wte_half, b_bcast)
                q = prefetched
                for i in range(len(q), min(LA, n_iters)):
                    q.append(stage_a(i))
                for it in range(n_iters):
                    if it + LA < n_iters:
                        q.append(stage_a(it + LA))
                    stage_b(it, q.popleft(), first=(it == 0))

            if "no_phase0" in variant:
                wte_lo = w_pool.tile([P, NCHUNK, 512], F16, tag="wte_lo")
                nc.vector.memset(wte_lo[:], 0.0)
                wte_hi = w_pool.tile([P, NCHUNK, 512], F16, tag="wte_hi")
                nc.vector.memset(wte_hi[:], 0.0)
                b_bcast = const_pool.tile([P, D], F32, tag="b_bcast")
                nc.vector.memset(b_bcast[:], 0.0)
                main_loop(TTILES * repeats, (wte_lo, wte_hi), b_bcast,
                          deque())
            elif full_rep:
                for rep in range(repeats):
                    # issue this pass's first x DMAs before the W_eff build
                    # so the ACT-ring copybacks don't delay the x stream
                    prefetched = deque(
                        stage_a(i) for i in range(min(LA, TTILES)))
                    wte_lo, wte_hi, b_bcast = phase0()
                    main_loop(TTILES, (wte_lo, wte_hi), b_bcast, prefetched)
            else:
                prefetched = deque(
                    stage_a(i) for i in range(min(LA, TTILES * repeats)))
                wte_lo, wte_hi, b_bcast = phase0()
                main_loop(TTILES * repeats, (wte_lo, wte_hi), b_bcast,
                          prefetched)

    nc.compile()
    return nc


def get_nc(repeats=1, variant=()):
    key = (repeats, tuple(variant))
    if key not in _NC_CACHE:
        _NC_CACHE[key] = _build_nc(repeats, variant)
    return _NC_CACHE[key]


def make_in_maps(x, t_scalar, W, b, A):
    f16 = np.float16
    x = np.asarray(x, dtype=np.float32)
    # [b, s, d] -> per core [tt, p, c, j] with s = tt*P+j, d = c*P+p
    xt = np.ascontiguousarray(
        x.reshape(B, TTILES, P, NCHUNK, P).transpose(0, 1, 4, 3, 2)
    ).astype(f16).reshape(B, TTILES, P, D)
    t = np.asarray(t_scalar, dtype=np.float32).reshape(1, 1)
    Wb = np.ascontiguousarray(np.asarray(W, dtype=np.float32)).astype(f16)
    b = np.ascontiguousarray(np.asarray(b, dtype=np.float32))
    Ab = np.ascontiguousarray(np.asarray(A, dtype=np.float32)).astype(f16)
    return [{"x": xt[i], "W": Wb, "b": b, "A": Ab, "t": t}
            for i in range(N_CORES)]


def kernel(x, t_scalar, W, b, A):
    nc = get_nc()
    in_maps = make_in_maps(x, t_scalar, W, b, A)
    res = bass_utils.run_bass_kernel_spmd(nc, in_maps,
                                          core_ids=list(range(N_CORES)))
    out = np.stack([res.results[i]["out"] for i in range(N_CORES)], axis=0)
    return np.ascontiguousarray(out.astype(np.float32))


if __name__ == "__main__":
    rng = np.random.default_rng(0)
    x = rng.standard_normal((B, S, D), dtype=np.float32)
    W = rng.standard_normal((D, D), dtype=np.float32) / 32.0
    b = rng.standard_normal((D,), dtype=np.float32) * 0.01
    A = rng.standard_normal((H, HD, HD), dtype=np.float32) * 0.02
    t = np.float32(0.6)
    out = kernel(x, t, W, b, A)
    print("out", out.shape, out.dtype)
